# revision 47
# baseline (speedup 1.0000x reference)
"""Query-axis-softmax attention on 8 trn2 cores.

Math (per head): scores = q @ k.T / sqrt(64); masked entries -> -1e9;
attn = softmax(scores, axis=QUERY); out = attn @ v.

Device layout trick: keep scores TRANSPOSED as [k, q].  Then
 - the softmax reduction (over q) is along the free axis -> fused into the
   scalar-engine Exp via accum_out,
 - the PV product is out.T[d, q] = sum_k v[k, d] * p[k, q] -> a plain matmul
   with contraction on the partition axis, no on-chip transposes at all.
Host pre-transposes q/k to [head, d, s] and the mask to [k, q] (as an
additive bias, folded into the scores via identity matmuls).

Heads are processed in PAIRS to pack the PE array:
 - QK^T per head contracts over only d=64 -> head A in array rows 0-63,
   head B in rows 64-127 (row tiling), concurrent.
 - the mask-add identity matmul splits into two K=64 row tiles (rows of I128),
   concurrent.
 - PV output per head is only 64 partitions -> head A in array cols 0-63,
   head B in cols 64-127 (col tiling), sharing one PSUM bank set.

Sharding: 32 (b*h) heads -> 4 per core, no cross-core communication.
"""

import numpy as np
import ml_dtypes

B, H, S, DK = 2, 16, 2048, 64
N_CORES = 8
HPC = (B * H) // N_CORES  # heads per core
P = 128                   # sbuf partitions
NSTRIP = S // P           # 16 strips of k-rows
HF = 1024                 # exp half-strip width (2 PSUM banks)
MASK_BIAS = -240.0        # exact in fp8e4; exp(0.125*(score-240)) < 4e-11

_CACHE = {}


def _build(mask_fp8=True, qk_pack=True, mask_pack=False, pv_pack=True, reps=1,
           no_exp=False, no_pv=False, dbl_exp=False, dbl_qk=False,
           qk_bf16=True, bufs_up=False, hi_exp=0, body_mult=1,
           no_mask=False, no_accum=False, sc_bufs=2, use_fp16=True,
           mask_dr=False, qk_dr=False, act_copy=False, out_sp=False,
           mask_first=False, mask_outside=False, mask_quad=True, hfw=HF):
    # mask_pack=True is a HW trap: two concurrent row-tiles draining into the
    # SAME psum bank collide on the bank write port (verified crash).  Row
    # tiles are only legal when they target different banks (like qk_pack).
    import concourse.tile as tile
    from concourse import mybir, bacc
    from concourse.masks import make_identity

    f32 = mybir.dt.float32
    f32r = mybir.dt.float32r
    bf16 = mybir.dt.bfloat16
    fp8 = mybir.dt.float8e4 if mask_fp8 else bf16
    Exp = mybir.ActivationFunctionType.Exp
    DR = mybir.MatmulPerfMode.DoubleRow

    f16 = mybir.dt.float16
    pdt = f16 if use_fp16 else bf16
    qk_dt = fp8 if qk_dr else ((f16 if use_fp16 else bf16) if qk_bf16 else f32r)
    nc = bacc.Bacc(None, target_bir_lowering=False)
    if qk_dr:
        # d split as (p, j): d = 2p + j, pairs streamed DoubleRow.
        qT = nc.dram_tensor("qT", [HPC, DK // 2, 2, S], qk_dt, kind="ExternalInput")
        kT = nc.dram_tensor("kT", [HPC, DK // 2, 2, S], qk_dt, kind="ExternalInput")
    else:
        qT = nc.dram_tensor("qT", [HPC, DK, S], qk_dt, kind="ExternalInput")
        kT = nc.dram_tensor("kT", [HPC, DK, S], qk_dt, kind="ExternalInput")
    v = nc.dram_tensor("v", [HPC, S, DK], pdt, kind="ExternalInput")
    if mask_dr:
        # strip rows k split as (p, j): k = 2p + j within each 128-row strip
        # for p < 64; partitions 64-127 are zero padding (DoubleRow appears
        # to require full-128-partition operands on HW).
        mT = nc.dram_tensor("mT", [NSTRIP, P, 2, S], fp8, kind="ExternalInput")
        idr = nc.dram_tensor("idr", [P, 2, P], fp8, kind="ExternalInput")
    else:
        mT = nc.dram_tensor("mT", [S, S], fp8, kind="ExternalInput")
        if mask_quad:
            # I64 replicated in both partition halves: the (64,64) quad's
            # weights must start at the same SB partition as its fmap.
            idr = nc.dram_tensor("idr", [P, P // 2], fp8, kind="ExternalInput")
    outT = nc.dram_tensor("outT", [HPC, DK, S], f32, kind="ExternalOutput")

    with tile.TileContext(nc) as tc:
        with (
            tc.tile_pool(name="mask", bufs=1) as mask_pool,
            tc.tile_pool(name="const", bufs=1) as const_pool,
            tc.tile_pool(name="qk", bufs=3 if bufs_up else 2) as qk_pool,
            tc.tile_pool(name="vload", bufs=3 if bufs_up else 2) as v_pool,
            tc.tile_pool(name="p", bufs=6 if bufs_up else 4) as p_pool,
            tc.tile_pool(name="small", bufs=16 if bufs_up else 8) as small_pool,
            tc.tile_pool(name="outsb", bufs=3 if bufs_up else 2) as out_pool,
            tc.tile_pool(name="scps", bufs=sc_bufs, space="PSUM") as sc_psum,
            tc.tile_pool(name="outps", bufs=1, space="PSUM") as out_psum_pool,
        ):
            if mask_dr:
                ident = const_pool.tile([P, 2, P], fp8)
            elif mask_quad:
                ident = const_pool.tile([P, P // 2], fp8)
            else:
                ident = const_pool.tile([P, P], fp8)
                make_identity(nc, ident[:])

            # Whole mask stays resident in SBUF (16 strips x [128, 2048] fp8,
            # or [64, 2, 2048] in DoubleRow pair layout).  Tiles allocated
            # here; DMAs issued inside the loop body (after the first pair's
            # q/k loads) so the first QK isn't queued behind 4MB of mask.
            mask_tiles = []
            for s in range(NSTRIP):
                if mask_dr:
                    mt = mask_pool.tile([P, 2, S], fp8, tag=f"m{s}")
                else:
                    mt = mask_pool.tile([P, S], fp8, tag=f"m{s}")
                mask_tiles.append(mt)

            def load_mask(s):
                if mask_dr:
                    nc.sync.dma_start(mask_tiles[s][:], mT[s])
                else:
                    nc.sync.dma_start(
                        mask_tiles[s][:], mT[s * P:(s + 1) * P, :]
                    )

            def qk_mask_exp(kview, qview, mstrip, s, hf, qh, pt, accum):
                """Scores for one [128k, hfw q] block of one head (rows half
                `hf` of the packed pair), then exp into pt with row-sum
                accumulation."""
                sc = sc_psum.tile([P, hfw], f32)
                for sub in range(hfw // 512):
                    cols = slice(sub * 512, (sub + 1) * 512)
                    q0 = qh * hfw + sub * 512

                    use_quad = mask_quad and not mask_dr and not no_mask
                    if use_quad:
                        # Identity is diagonal: out[half] only needs mask
                        # rows [half] -> two 64-contraction matmuls in
                        # disjoint array quadrants (0,0)/(64,64), concurrent,
                        # writing disjoint partition halves of the same bank.
                        # ORDER vs the head's QK matters: the PE overlaps
                        # array-disjoint instructions, and two in-flight
                        # matmuls must never co-write a bank.  Head A's QK
                        # (rows 0-63) conflicts with quad (0,0) -> QK first
                        # serializes.  Head B's QK (rows 64-127) conflicts
                        # with quad (64,64) -> quads first, QK last.
                        mask_before_qk = hf == 1
                    else:
                        mask_before_qk = mask_first and mask_dr

                    def emit_quads(flags):
                        # flags: ((start, stop, skip) for quad0, quad1).
                        # HW: start clears the whole bank's has_written;
                        # nostart overwrites unwritten rows / accumulates
                        # written ones.  Sim group tracker is partition-
                        # offset-blind, hence the skip/stop contortions.
                        (s0, t0, k0), (s1, t1, k1) = flags
                        nc.tensor.matmul(
                            sc[0:64, cols],
                            lhsT=ident[0:64, :],
                            rhs=mstrip[0:64, q0:q0 + 512],
                            start=s0,
                            stop=t0,
                            tile_position=(0, 0),
                            skip_group_check=k0,
                        )
                        nc.tensor.matmul(
                            sc[64:128, cols],
                            lhsT=ident[64:128, :],
                            rhs=mstrip[64:128, q0:q0 + 512],
                            start=s1,
                            stop=t1,
                            tile_position=(64, 64),
                            skip_group_check=k1,
                        )

                    def emit_qk(start, stop, skip=False):
                        if qk_dr:
                            nc.tensor.matmul(
                                sc[:, cols],
                                lhsT=kview[:, :, s * P:(s + 1) * P],
                                rhs=qview[:, :, q0:q0 + 512],
                                start=start,
                                stop=stop,
                                perf_mode=DR,
                                skip_group_check=skip,
                            )
                        else:
                          for dup in range(2 if dbl_qk else 1):
                            nc.tensor.matmul(
                                sc[:, cols],
                                lhsT=kview[:, s * P:(s + 1) * P],
                                rhs=qview[:, q0:q0 + 512],
                                start=start,
                                stop=stop,
                                tile_position=(64 * hf, 0) if qk_pack else None,
                                skip_group_check=skip,
                            )

                    # All sc-group matmuls skip the sim's psum group tracker:
                    # it assumes a 2KB partition stride and mis-addresses
                    # partition-subset instructions on this 2-bank tile.  The
                    # interp's per-partition pending-zero data path (which
                    # models HW has_written) uses real strides and is what
                    # start/stop semantics rely on.
                    if use_quad and hf == 0:
                        # QK (rows 0-63) first: quad (0,0) array-conflicts
                        # with it -> serializes; quads then run concurrently.
                        emit_qk(True, False, skip=True)
                        emit_quads(((False, True, True), (False, True, True)))
                    elif use_quad:
                        # head B QK is rows 64-127: quads go FIRST so the
                        # (64,64) quad array-conflicts with QK -> serializes.
                        emit_quads(((True, False, True), (True, False, True)))
                        emit_qk(False, True, skip=True)
                    elif mask_before_qk and not no_mask:
                        nc.tensor.matmul(
                            sc[:, cols],
                            lhsT=ident[:],
                            rhs=mstrip[:, :, q0:q0 + 512],
                            start=True,
                            stop=False,
                            perf_mode=DR,
                        )
                        emit_qk(False, True)
                    elif no_mask:
                        emit_qk(True, True)
                    elif mask_dr:
                        emit_qk(True, False)
                        nc.tensor.matmul(
                            sc[:, cols],
                            lhsT=ident[:],
                            rhs=mstrip[:, :, q0:q0 + 512],
                            start=False,
                            stop=True,
                            perf_mode=DR,
                        )
                    elif mask_pack:
                        emit_qk(True, False)
                        nc.tensor.matmul(
                            sc[:, cols],
                            lhsT=ident[0:64, :],
                            rhs=mstrip[0:64, q0:q0 + 512],
                            start=False,
                            stop=False,
                            tile_position=(0, 0),
                        )
                        nc.tensor.matmul(
                            sc[:, cols],
                            lhsT=ident[64:128, :],
                            rhs=mstrip[64:128, q0:q0 + 512],
                            start=False,
                            stop=True,
                            tile_position=(64, 0),
                        )
                    else:
                        emit_qk(True, False)
                        nc.tensor.matmul(
                            sc[:, cols],
                            lhsT=ident[:],
                            rhs=mstrip[:, q0:q0 + 512],
                            start=False,
                            stop=True,
                        )
                if not no_exp:
                    import contextlib
                    prio = (
                        tc.high_priority(hi_exp) if hi_exp
                        else contextlib.nullcontext()
                    )
                    if dbl_exp:
                        scratch = p_pool.tile([P, HF], pdt, tag="expscratch")
                        nc.scalar.activation(
                            out=scratch[:], in_=sc[:], func=Exp, scale=0.125
                        )
                    with prio:
                        nc.scalar.activation(
                            out=pt, in_=sc[:], func=Exp, scale=0.125,
                            accum_out=None if no_accum else accum,
                        )
                    if no_accum:
                        nc.vector.memset(accum, 1.0)

            import contextlib

            if mask_outside:
                if mask_dr or mask_quad:
                    nc.sync.dma_start(ident[:], idr[:])
                for s in range(NSTRIP):
                    load_mask(s)

            loop_cm = (
                tc.For_i(0, reps, 1) if reps > 1 else contextlib.nullcontext()
            )
            with loop_cm:
              for hp in [x for _ in range(body_mult) for x in range(HPC // 2)]:
                hA, hB = 2 * hp, 2 * hp + 1
                if qk_dr:
                    qtA = qk_pool.tile([DK // 2, 2, S], qk_dt, tag="qA")
                    qtB = qk_pool.tile([DK // 2, 2, S], qk_dt, tag="qB")
                    ktA = qk_pool.tile([DK // 2, 2, S], qk_dt, tag="kA")
                    ktB = qk_pool.tile([DK // 2, 2, S], qk_dt, tag="kB")
                    qtviews = [qtA[:], qtB[:]]
                    ktviews = [ktA[:], ktB[:]]
                elif qk_pack:
                    qts = qk_pool.tile([P, S], qk_dt, tag="q")
                    kts = qk_pool.tile([P, S], qk_dt, tag="k")
                    qtviews = [qts[0:DK, :], qts[DK:P, :]]
                    ktviews = [kts[0:DK, :], kts[DK:P, :]]
                else:
                    qtA = qk_pool.tile([DK, S], qk_dt, tag="qA")
                    qtB = qk_pool.tile([DK, S], qk_dt, tag="qB")
                    ktA = qk_pool.tile([DK, S], qk_dt, tag="kA")
                    ktB = qk_pool.tile([DK, S], qk_dt, tag="kB")
                    qtviews = [qtA[:], qtB[:]]
                    ktviews = [ktA[:], ktB[:]]
                vts = v_pool.tile([P, 2, NSTRIP, DK], pdt, tag="v")
                if hp == 0 and mask_outside:
                    nc.sync.dma_start(ktviews[0], kT[hA])
                    nc.sync.dma_start(qtviews[0], qT[hA])
                    nc.sync.dma_start(ktviews[1], kT[hB])
                    nc.sync.dma_start(qtviews[1], qT[hB])
                    nc.sync.dma_start(
                        vts[:, 0], v[hA].rearrange("(s p) d -> p s d", p=P)
                    )
                    nc.sync.dma_start(
                        vts[:, 1], v[hB].rearrange("(s p) d -> p s d", p=P)
                    )
                elif hp == 0:
                    # DMA issue order by first-use time: idr+kA+qA+mask0
                    # feed the first fill (~1us), mask[s] isn't needed until
                    # ~5.3*s us, v not before the first PV (~10us).
                    if mask_dr or mask_quad:
                        nc.sync.dma_start(ident[:], idr[:])
                    nc.sync.dma_start(ktviews[0], kT[hA])
                    nc.sync.dma_start(qtviews[0], qT[hA])
                    load_mask(0)
                    nc.sync.dma_start(ktviews[1], kT[hB])
                    nc.sync.dma_start(qtviews[1], qT[hB])
                    load_mask(1)
                    nc.sync.dma_start(
                        vts[:, 0], v[hA].rearrange("(s p) d -> p s d", p=P)
                    )
                    nc.sync.dma_start(
                        vts[:, 1], v[hB].rearrange("(s p) d -> p s d", p=P)
                    )
                    for s in range(2, NSTRIP):
                        load_mask(s)
                else:
                    nc.sync.dma_start(ktviews[0], kT[hA])
                    nc.sync.dma_start(qtviews[0], qT[hA])
                    nc.sync.dma_start(ktviews[1], kT[hB])
                    nc.sync.dma_start(qtviews[1], qT[hB])
                    nc.sync.dma_start(
                        vts[:, 0], v[hA].rearrange("(s p) d -> p s d", p=P)
                    )
                    nc.sync.dma_start(
                        vts[:, 1], v[hB].rearrange("(s p) d -> p s d", p=P)
                    )

                out_ps = (
                    None if (no_exp or no_pv)
                    else out_psum_pool.tile([P, S], f32)
                )

                def emit_pv_head(s, hf, pt, ssum):
                    """Normalize v rows by strip-s row sums and accumulate
                    out.T += vsc.T @ p for one head of the pair."""
                    if (hf == 1 and not pv_pack) or no_exp or no_pv:
                        return
                    stot = small_pool.tile([P, 1], f32, tag=f"stot{hf}")
                    if S // hfw == 2:
                        nc.vector.tensor_add(
                            stot[:], ssum[:, 0:1], ssum[:, 1:2]
                        )
                    else:
                        nc.vector.tensor_reduce(
                            stot[:], ssum[:],
                            axis=mybir.AxisListType.X,
                            op=mybir.AluOpType.add,
                        )
                    sinv = small_pool.tile([P, 1], f32, tag=f"sinv{hf}")
                    nc.vector.reciprocal(sinv[:], stot[:])
                    vsc = small_pool.tile([P, DK], pdt, tag=f"vsc{hf}")
                    nc.vector.tensor_scalar_mul(
                        vsc[:], vts[:, hf, s, :], sinv[:]
                    )
                    for qc in range(4):
                        cols = slice(qc * 512, (qc + 1) * 512)
                        nc.tensor.matmul(
                            out_ps[64 * hf:64 * (hf + 1), cols],
                            lhsT=vsc[:],
                            rhs=pt[:, cols],
                            start=(s == 0),
                            stop=(s == NSTRIP - 1),
                            tile_position=(0, 64 * hf),
                            # A/B col-tiles share the bank but write
                            # disjoint partition halves; the sim's group
                            # check is address-only and would reject it.
                            skip_group_check=True,
                        )

                # Software-pipelined: PV for strip s-1's head hf is emitted
                # right after strip s's head-hf score fills, so the in-order
                # PE queue always has the next exp's scores filled before
                # the (longer) PV work.
                pend = [None, None]
                for s in range(NSTRIP):
                    mstrip = mask_tiles[s]
                    for hf in range(2):  # head A=0 / head B=1 of the pair
                        pt = p_pool.tile([P, S], pdt, tag=f"p{hf}")
                        nq = S // hfw
                        ssum = small_pool.tile([P, nq], f32, tag=f"ssum{hf}")
                        for qh in range(nq):  # q chunks
                            qk_mask_exp(
                                ktviews[hf],
                                qtviews[hf],
                                mstrip,
                                s,
                                hf,
                                qh,
                                pt[:, qh * hfw:(qh + 1) * hfw],
                                ssum[:, qh:qh + 1],
                            )
                        if pend[hf] is not None:
                            emit_pv_head(*pend[hf])
                        pend[hf] = (s, hf, pt, ssum)
                for hf in range(2):
                    emit_pv_head(*pend[hf])
                if out_ps is None:
                    out_sb = out_pool.tile([P, S], f32)
                    nc.vector.memset(out_sb[:], 0.0)
                    nc.gpsimd.dma_start(outT[hA], out_sb[0:DK, :])
                    nc.gpsimd.dma_start(outT[hB], out_sb[DK:P, :])
                else:
                    # Chunked copy+DMA so the tail pipelines; on the last
                    # pair the second copy runs on ACT (idle by then) and
                    # the DMA issues on the idle SP hwdge queue instead of
                    # the ~1us/issue gpsimd swdge path.
                    out_sb = out_pool.tile([P, S], f32)
                    last = hp == HPC // 2 - 1
                    dma_eng = nc.sync if (last and out_sp) else nc.gpsimd
                    for qc in range(2):
                        cols = slice(qc * 1024, (qc + 1) * 1024)
                        if last and qc == 1 and act_copy:
                            nc.scalar.copy(out_sb[:, cols], out_ps[:, cols])
                        else:
                            nc.vector.tensor_copy(
                                out_sb[:, cols], out_ps[:, cols]
                            )
                        dma_eng.dma_start(outT[hA][:, cols], out_sb[0:DK, cols])
                        dma_eng.dma_start(outT[hB][:, cols], out_sb[DK:P, cols])

    nc.compile()
    return nc


def get_nc(**opts):
    key = tuple(sorted(opts.items()))
    if key not in _CACHE:
        _CACHE[key] = _build(**opts)
    return _CACHE[key]


def make_in_maps(q, k, v, mask, mask_fp8=True, qk_bf16=True,
                 use_fp16=True, mask_dr=False, qk_dr=False, mask_quad=True):
    """Full inputs -> list of 8 per-core input maps."""
    q32 = np.asarray(q, np.float32).reshape(B * H, S, DK)
    k32 = np.asarray(k, np.float32).reshape(B * H, S, DK)
    vdt = np.float16 if use_fp16 else ml_dtypes.bfloat16
    v32 = np.ascontiguousarray(np.asarray(v, vdt).reshape(B * H, S, DK))
    if qk_dr:
        qkdt = ml_dtypes.float8_e4m3
    else:
        qkdt = (np.float16 if use_fp16 else ml_dtypes.bfloat16) if qk_bf16 else np.float32
    qT = np.ascontiguousarray(q32.transpose(0, 2, 1)).astype(qkdt)
    kT = np.ascontiguousarray(k32.transpose(0, 2, 1)).astype(qkdt)
    if qk_dr:
        # [h, d, s] -> [h, d//2, 2, s], d = 2p + j
        qT = qT.reshape(B * H, DK // 2, 2, S)
        kT = kT.reshape(B * H, DK // 2, 2, S)
    maskT = np.asarray(mask).reshape(S, S).T            # [k, q]
    mdt = ml_dtypes.float8_e4m3 if mask_fp8 else ml_dtypes.bfloat16
    mTb = np.where(maskT, np.float32(MASK_BIAS), np.float32(0.0)).astype(mdt)
    idr = None
    if mask_dr:
        # [k, q] -> [strip, p, j, q] with k = 128*strip + 2p + j for p < 64;
        # partitions 64-127 zero-padded (HW DoubleRow wants 128 partitions).
        mT_dr = np.zeros((NSTRIP, P, 2, S), mdt)
        mT_dr[:, : P // 2] = mTb.reshape(NSTRIP, P // 2, 2, S)
        mTb = mT_dr
        idr = np.zeros((P, 2, P), mdt)
        for p in range(P // 2):
            for j in range(2):
                idr[p, j, 2 * p + j] = 1.0
    elif mask_quad:
        i64 = np.eye(P // 2).astype(mdt)
        idr = np.concatenate([i64, i64], axis=0)
    in_maps = []
    for c in range(N_CORES):
        sl = slice(c * HPC, (c + 1) * HPC)
        m = {
            "qT": np.ascontiguousarray(qT[sl]),
            "kT": np.ascontiguousarray(kT[sl]),
            "v": v32[sl],
            "mT": mTb,
        }
        if idr is not None:
            m["idr"] = idr
        in_maps.append(m)
    return in_maps


def assemble_out(per_core_outT):
    """8 x [HPC, DK, S] -> [B, H, S, DK]."""
    out = np.concatenate([np.asarray(o) for o in per_core_outT], axis=0)
    return np.ascontiguousarray(
        out.reshape(B, H, DK, S).transpose(0, 1, 3, 2)
    ).astype(np.float32)


def kernel(q, k, v, mask):
    from concourse.bass_utils import run_bass_kernel_spmd

    nc = get_nc()
    in_maps = make_in_maps(q, k, v, mask)
    res = run_bass_kernel_spmd(nc, in_maps, core_ids=list(range(N_CORES)))
    return assemble_out([r["outT"] for r in res.results])



# revision 48
# speedup vs baseline: 1.1071x; 1.1071x over previous
"""Query-axis-softmax attention on 8 trn2 cores.

Math (per head): scores = q @ k.T / sqrt(64); masked entries -> -1e9;
attn = softmax(scores, axis=QUERY); out = attn @ v.

Device layout trick: keep scores TRANSPOSED as [k, q].  Then
 - the softmax reduction (over q) is along the free axis -> fused into the
   scalar-engine Exp via accum_out,
 - the PV product is out.T[d, q] = sum_k v[k, d] * p[k, q] -> a plain matmul
   with contraction on the partition axis, no on-chip transposes at all.
Host pre-transposes q/k to [head, d, s] and the mask to [k, q] (as an
additive bias, folded into the scores via identity matmuls).

Heads are processed in PAIRS to pack the PE array:
 - QK^T per head contracts over only d=64 -> head A in array rows 0-63,
   head B in rows 64-127 (row tiling), concurrent.
 - the mask-add identity matmul splits into two K=64 row tiles (rows of I128),
   concurrent.
 - PV output per head is only 64 partitions -> head A in array cols 0-63,
   head B in cols 64-127 (col tiling), sharing one PSUM bank set.

Sharding: 32 (b*h) heads -> 4 per core, no cross-core communication.
"""

import numpy as np
import ml_dtypes

B, H, S, DK = 2, 16, 2048, 64
N_CORES = 8
HPC = (B * H) // N_CORES  # heads per core
P = 128                   # sbuf partitions
NSTRIP = S // P           # 16 strips of k-rows
HF = 1024                 # exp half-strip width (2 PSUM banks)
MASK_BIAS = -240.0        # exact in fp8e4; exp(0.125*(score-240)) < 4e-11

_CACHE = {}


def _build(mask_fp8=True, qk_pack=True, mask_pack=False, pv_pack=True, reps=1,
           no_exp=False, no_pv=False, dbl_exp=False, dbl_qk=False,
           qk_bf16=True, bufs_up=False, hi_exp=0, body_mult=1,
           no_mask=False, no_accum=False, sc_bufs=2, use_fp16=True,
           mask_dr=False, qk_dr=False, act_copy=False, out_sp=False,
           mask_first=False, mask_outside=False, mask_quad=True, hfw=HF):
    # mask_pack=True is a HW trap: two concurrent row-tiles draining into the
    # SAME psum bank collide on the bank write port (verified crash).  Row
    # tiles are only legal when they target different banks (like qk_pack).
    import concourse.tile as tile
    from concourse import mybir, bacc
    from concourse.masks import make_identity

    f32 = mybir.dt.float32
    f32r = mybir.dt.float32r
    bf16 = mybir.dt.bfloat16
    fp8 = mybir.dt.float8e4 if mask_fp8 else bf16
    Exp = mybir.ActivationFunctionType.Exp
    DR = mybir.MatmulPerfMode.DoubleRow

    f16 = mybir.dt.float16
    pdt = f16 if use_fp16 else bf16
    qk_dt = fp8 if qk_dr else ((f16 if use_fp16 else bf16) if qk_bf16 else f32r)
    nc = bacc.Bacc(None, target_bir_lowering=False)
    if qk_dr:
        # d split as (p, j): d = 2p + j, pairs streamed DoubleRow.
        qT = nc.dram_tensor("qT", [HPC, DK // 2, 2, S], qk_dt, kind="ExternalInput")
        kT = nc.dram_tensor("kT", [HPC, DK // 2, 2, S], qk_dt, kind="ExternalInput")
    else:
        qT = nc.dram_tensor("qT", [HPC, DK, S], qk_dt, kind="ExternalInput")
        kT = nc.dram_tensor("kT", [HPC, DK, S], qk_dt, kind="ExternalInput")
    # v pre-arranged host-side to [head, p, strip, d] so the DMA is one
    # contiguous 2KB run per partition (the old "(s p) d -> p s d" rearrange
    # generated 128-byte strided descriptors).
    v = nc.dram_tensor("v", [HPC, P, NSTRIP, DK], pdt, kind="ExternalInput")
    if mask_dr:
        # strip rows k split as (p, j): k = 2p + j within each 128-row strip
        # for p < 64; partitions 64-127 are zero padding (DoubleRow appears
        # to require full-128-partition operands on HW).
        mT = nc.dram_tensor("mT", [NSTRIP, P, 2, S], fp8, kind="ExternalInput")
        idr = nc.dram_tensor("idr", [P, 2, P], fp8, kind="ExternalInput")
    else:
        mT = nc.dram_tensor("mT", [S, S], fp8, kind="ExternalInput")
        if mask_quad:
            # I64 replicated in both partition halves: the (64,64) quad's
            # weights must start at the same SB partition as its fmap.
            idr = nc.dram_tensor("idr", [P, P // 2], fp8, kind="ExternalInput")
    outT = nc.dram_tensor("outT", [HPC, DK, S], f32, kind="ExternalOutput")

    with tile.TileContext(nc) as tc:
        with (
            tc.tile_pool(name="mask", bufs=1) as mask_pool,
            tc.tile_pool(name="const", bufs=1) as const_pool,
            tc.tile_pool(name="qk", bufs=3 if bufs_up else 2) as qk_pool,
            tc.tile_pool(name="vload", bufs=3 if bufs_up else 2) as v_pool,
            tc.tile_pool(name="p", bufs=6 if bufs_up else 4) as p_pool,
            tc.tile_pool(name="small", bufs=16 if bufs_up else 8) as small_pool,
            tc.tile_pool(name="outsb", bufs=3 if bufs_up else 2) as out_pool,
            tc.tile_pool(name="scps", bufs=sc_bufs, space="PSUM") as sc_psum,
            tc.tile_pool(name="outps", bufs=1, space="PSUM") as out_psum_pool,
        ):
            if mask_dr:
                ident = const_pool.tile([P, 2, P], fp8)
            elif mask_quad:
                ident = const_pool.tile([P, P // 2], fp8)
            else:
                ident = const_pool.tile([P, P], fp8)
                make_identity(nc, ident[:])

            # Whole mask stays resident in SBUF (16 strips x [128, 2048] fp8,
            # or [64, 2, 2048] in DoubleRow pair layout).  Tiles allocated
            # here; DMAs issued inside the loop body (after the first pair's
            # q/k loads) so the first QK isn't queued behind 4MB of mask.
            mask_tiles = []
            for s in range(NSTRIP):
                if mask_dr:
                    mt = mask_pool.tile([P, 2, S], fp8, tag=f"m{s}")
                else:
                    mt = mask_pool.tile([P, S], fp8, tag=f"m{s}")
                mask_tiles.append(mt)

            def load_mask(s):
                if mask_dr:
                    nc.sync.dma_start(mask_tiles[s][:], mT[s])
                else:
                    nc.sync.dma_start(
                        mask_tiles[s][:], mT[s * P:(s + 1) * P, :]
                    )

            def qk_mask_exp(kview, qview, mstrip, s, hf, qh, pt, accum):
                """Scores for one [128k, hfw q] block of one head (rows half
                `hf` of the packed pair), then exp into pt with row-sum
                accumulation."""
                sc = sc_psum.tile([P, hfw], f32)
                for sub in range(hfw // 512):
                    cols = slice(sub * 512, (sub + 1) * 512)
                    q0 = qh * hfw + sub * 512

                    use_quad = mask_quad and not mask_dr and not no_mask
                    if use_quad:
                        # Identity is diagonal: out[half] only needs mask
                        # rows [half] -> two 64-contraction matmuls in
                        # disjoint array quadrants (0,0)/(64,64), concurrent,
                        # writing disjoint partition halves of the same bank.
                        # ORDER vs the head's QK matters: the PE overlaps
                        # array-disjoint instructions, and two in-flight
                        # matmuls must never co-write a bank.  Head A's QK
                        # (rows 0-63) conflicts with quad (0,0) -> QK first
                        # serializes.  Head B's QK (rows 64-127) conflicts
                        # with quad (64,64) -> quads first, QK last.
                        mask_before_qk = hf == 1
                    else:
                        mask_before_qk = mask_first and mask_dr

                    def emit_quads(flags):
                        # flags: ((start, stop, skip) for quad0, quad1).
                        # HW: start clears the whole bank's has_written;
                        # nostart overwrites unwritten rows / accumulates
                        # written ones.  Sim group tracker is partition-
                        # offset-blind, hence the skip/stop contortions.
                        (s0, t0, k0), (s1, t1, k1) = flags
                        nc.tensor.matmul(
                            sc[0:64, cols],
                            lhsT=ident[0:64, :],
                            rhs=mstrip[0:64, q0:q0 + 512],
                            start=s0,
                            stop=t0,
                            tile_position=(0, 0),
                            skip_group_check=k0,
                        )
                        nc.tensor.matmul(
                            sc[64:128, cols],
                            lhsT=ident[64:128, :],
                            rhs=mstrip[64:128, q0:q0 + 512],
                            start=s1,
                            stop=t1,
                            tile_position=(64, 64),
                            skip_group_check=k1,
                        )

                    def emit_qk(start, stop, skip=False):
                        if qk_dr:
                            nc.tensor.matmul(
                                sc[:, cols],
                                lhsT=kview[:, :, s * P:(s + 1) * P],
                                rhs=qview[:, :, q0:q0 + 512],
                                start=start,
                                stop=stop,
                                perf_mode=DR,
                                skip_group_check=skip,
                            )
                        else:
                          for dup in range(2 if dbl_qk else 1):
                            nc.tensor.matmul(
                                sc[:, cols],
                                lhsT=kview[:, s * P:(s + 1) * P],
                                rhs=qview[:, q0:q0 + 512],
                                start=start,
                                stop=stop,
                                tile_position=(64 * hf, 0) if qk_pack else None,
                                skip_group_check=skip,
                            )

                    # All sc-group matmuls skip the sim's psum group tracker:
                    # it assumes a 2KB partition stride and mis-addresses
                    # partition-subset instructions on this 2-bank tile.  The
                    # interp's per-partition pending-zero data path (which
                    # models HW has_written) uses real strides and is what
                    # start/stop semantics rely on.
                    if use_quad and hf == 0:
                        # QK (rows 0-63) first: quad (0,0) array-conflicts
                        # with it -> serializes; quads then run concurrently.
                        emit_qk(True, False, skip=True)
                        emit_quads(((False, True, True), (False, True, True)))
                    elif use_quad:
                        # head B QK is rows 64-127: quads go FIRST so the
                        # (64,64) quad array-conflicts with QK -> serializes.
                        emit_quads(((True, False, True), (True, False, True)))
                        emit_qk(False, True, skip=True)
                    elif mask_before_qk and not no_mask:
                        nc.tensor.matmul(
                            sc[:, cols],
                            lhsT=ident[:],
                            rhs=mstrip[:, :, q0:q0 + 512],
                            start=True,
                            stop=False,
                            perf_mode=DR,
                        )
                        emit_qk(False, True)
                    elif no_mask:
                        emit_qk(True, True)
                    elif mask_dr:
                        emit_qk(True, False)
                        nc.tensor.matmul(
                            sc[:, cols],
                            lhsT=ident[:],
                            rhs=mstrip[:, :, q0:q0 + 512],
                            start=False,
                            stop=True,
                            perf_mode=DR,
                        )
                    elif mask_pack:
                        emit_qk(True, False)
                        nc.tensor.matmul(
                            sc[:, cols],
                            lhsT=ident[0:64, :],
                            rhs=mstrip[0:64, q0:q0 + 512],
                            start=False,
                            stop=False,
                            tile_position=(0, 0),
                        )
                        nc.tensor.matmul(
                            sc[:, cols],
                            lhsT=ident[64:128, :],
                            rhs=mstrip[64:128, q0:q0 + 512],
                            start=False,
                            stop=True,
                            tile_position=(64, 0),
                        )
                    else:
                        emit_qk(True, False)
                        nc.tensor.matmul(
                            sc[:, cols],
                            lhsT=ident[:],
                            rhs=mstrip[:, q0:q0 + 512],
                            start=False,
                            stop=True,
                        )
                if not no_exp:
                    import contextlib
                    prio = (
                        tc.high_priority(hi_exp) if hi_exp
                        else contextlib.nullcontext()
                    )
                    if dbl_exp:
                        scratch = p_pool.tile([P, HF], pdt, tag="expscratch")
                        nc.scalar.activation(
                            out=scratch[:], in_=sc[:], func=Exp, scale=0.125
                        )
                    with prio:
                        nc.scalar.activation(
                            out=pt, in_=sc[:], func=Exp, scale=0.125,
                            accum_out=None if no_accum else accum,
                        )
                    if no_accum:
                        nc.vector.memset(accum, 1.0)

            import contextlib

            if mask_outside:
                if mask_dr or mask_quad:
                    nc.sync.dma_start(ident[:], idr[:])
                for s in range(NSTRIP):
                    load_mask(s)

            loop_cm = (
                tc.For_i(0, reps, 1) if reps > 1 else contextlib.nullcontext()
            )
            with loop_cm:
              for hp in [x for _ in range(body_mult) for x in range(HPC // 2)]:
                hA, hB = 2 * hp, 2 * hp + 1
                if qk_dr:
                    qtA = qk_pool.tile([DK // 2, 2, S], qk_dt, tag="qA")
                    qtB = qk_pool.tile([DK // 2, 2, S], qk_dt, tag="qB")
                    ktA = qk_pool.tile([DK // 2, 2, S], qk_dt, tag="kA")
                    ktB = qk_pool.tile([DK // 2, 2, S], qk_dt, tag="kB")
                    qtviews = [qtA[:], qtB[:]]
                    ktviews = [ktA[:], ktB[:]]
                elif qk_pack:
                    qts = qk_pool.tile([P, S], qk_dt, tag="q")
                    kts = qk_pool.tile([P, S], qk_dt, tag="k")
                    qtviews = [qts[0:DK, :], qts[DK:P, :]]
                    ktviews = [kts[0:DK, :], kts[DK:P, :]]
                else:
                    qtA = qk_pool.tile([DK, S], qk_dt, tag="qA")
                    qtB = qk_pool.tile([DK, S], qk_dt, tag="qB")
                    ktA = qk_pool.tile([DK, S], qk_dt, tag="kA")
                    ktB = qk_pool.tile([DK, S], qk_dt, tag="kB")
                    qtviews = [qtA[:], qtB[:]]
                    ktviews = [ktA[:], ktB[:]]
                vts = v_pool.tile([P, 2, NSTRIP, DK], pdt, tag="v")
                if hp == 0 and mask_outside:
                    nc.sync.dma_start(ktviews[0], kT[hA])
                    nc.sync.dma_start(qtviews[0], qT[hA])
                    nc.sync.dma_start(ktviews[1], kT[hB])
                    nc.sync.dma_start(qtviews[1], qT[hB])
                    nc.sync.dma_start(vts[:, 0], v[hA])
                    nc.sync.dma_start(vts[:, 1], v[hB])
                elif hp == 0:
                    # DMA issue order by first-use time: idr+kA+qA+mask0
                    # feed the first fill (~1us), mask[s] isn't needed until
                    # ~5.3*s us, v not before the first PV (~10us).
                    if mask_dr or mask_quad:
                        nc.sync.dma_start(ident[:], idr[:])
                    nc.sync.dma_start(ktviews[0], kT[hA])
                    nc.sync.dma_start(qtviews[0], qT[hA])
                    load_mask(0)
                    nc.sync.dma_start(ktviews[1], kT[hB])
                    nc.sync.dma_start(qtviews[1], qT[hB])
                    load_mask(1)
                    nc.sync.dma_start(vts[:, 0], v[hA])
                    nc.sync.dma_start(vts[:, 1], v[hB])
                    for s in range(2, NSTRIP):
                        load_mask(s)
                else:
                    nc.sync.dma_start(ktviews[0], kT[hA])
                    nc.sync.dma_start(qtviews[0], qT[hA])
                    nc.sync.dma_start(ktviews[1], kT[hB])
                    nc.sync.dma_start(qtviews[1], qT[hB])
                    nc.sync.dma_start(vts[:, 0], v[hA])
                    nc.sync.dma_start(vts[:, 1], v[hB])

                out_ps = (
                    None if (no_exp or no_pv)
                    else out_psum_pool.tile([P, S], f32)
                )

                def emit_pv_head(s, hf, pt, ssum):
                    """Normalize v rows by strip-s row sums and accumulate
                    out.T += vsc.T @ p for one head of the pair."""
                    if (hf == 1 and not pv_pack) or no_exp or no_pv:
                        return
                    stot = small_pool.tile([P, 1], f32, tag=f"stot{hf}")
                    if S // hfw == 2:
                        nc.vector.tensor_add(
                            stot[:], ssum[:, 0:1], ssum[:, 1:2]
                        )
                    else:
                        nc.vector.tensor_reduce(
                            stot[:], ssum[:],
                            axis=mybir.AxisListType.X,
                            op=mybir.AluOpType.add,
                        )
                    sinv = small_pool.tile([P, 1], f32, tag=f"sinv{hf}")
                    nc.vector.reciprocal(sinv[:], stot[:])
                    vsc = small_pool.tile([P, DK], pdt, tag=f"vsc{hf}")
                    nc.vector.tensor_scalar_mul(
                        vsc[:], vts[:, hf, s, :], sinv[:]
                    )
                    for qc in range(4):
                        cols = slice(qc * 512, (qc + 1) * 512)
                        nc.tensor.matmul(
                            out_ps[64 * hf:64 * (hf + 1), cols],
                            lhsT=vsc[:],
                            rhs=pt[:, cols],
                            start=(s == 0),
                            stop=(s == NSTRIP - 1),
                            tile_position=(0, 64 * hf),
                            # A/B col-tiles share the bank but write
                            # disjoint partition halves; the sim's group
                            # check is address-only and would reject it.
                            skip_group_check=True,
                        )

                # Software-pipelined: PV for strip s-1's head hf is emitted
                # right after strip s's head-hf score fills, so the in-order
                # PE queue always has the next exp's scores filled before
                # the (longer) PV work.
                pend = [None, None]
                for s in range(NSTRIP):
                    mstrip = mask_tiles[s]
                    for hf in range(2):  # head A=0 / head B=1 of the pair
                        pt = p_pool.tile([P, S], pdt, tag=f"p{hf}")
                        nq = S // hfw
                        ssum = small_pool.tile([P, nq], f32, tag=f"ssum{hf}")
                        for qh in range(nq):  # q chunks
                            qk_mask_exp(
                                ktviews[hf],
                                qtviews[hf],
                                mstrip,
                                s,
                                hf,
                                qh,
                                pt[:, qh * hfw:(qh + 1) * hfw],
                                ssum[:, qh:qh + 1],
                            )
                        if pend[hf] is not None:
                            emit_pv_head(*pend[hf])
                        pend[hf] = (s, hf, pt, ssum)
                for hf in range(2):
                    emit_pv_head(*pend[hf])
                if out_ps is None:
                    out_sb = out_pool.tile([P, S], f32)
                    nc.vector.memset(out_sb[:], 0.0)
                    nc.gpsimd.dma_start(outT[hA], out_sb[0:DK, :])
                    nc.gpsimd.dma_start(outT[hB], out_sb[DK:P, :])
                else:
                    # Chunked copy+DMA so the tail pipelines; on the last
                    # pair the second copy runs on ACT (idle by then) and
                    # the DMA issues on the idle SP hwdge queue instead of
                    # the ~1us/issue gpsimd swdge path.
                    out_sb = out_pool.tile([P, S], f32)
                    last = hp == HPC // 2 - 1
                    dma_eng = nc.sync if (last and out_sp) else nc.gpsimd
                    for qc in range(2):
                        cols = slice(qc * 1024, (qc + 1) * 1024)
                        if last and qc == 1 and act_copy:
                            nc.scalar.copy(out_sb[:, cols], out_ps[:, cols])
                        else:
                            nc.vector.tensor_copy(
                                out_sb[:, cols], out_ps[:, cols]
                            )
                        dma_eng.dma_start(outT[hA][:, cols], out_sb[0:DK, cols])
                        dma_eng.dma_start(outT[hB][:, cols], out_sb[DK:P, cols])

    nc.compile()
    return nc


def get_nc(**opts):
    key = tuple(sorted(opts.items()))
    if key not in _CACHE:
        _CACHE[key] = _build(**opts)
    return _CACHE[key]


def make_in_maps(q, k, v, mask, mask_fp8=True, qk_bf16=True,
                 use_fp16=True, mask_dr=False, qk_dr=False, mask_quad=True):
    """Full inputs -> list of 8 per-core input maps."""
    q32 = np.asarray(q, np.float32).reshape(B * H, S, DK)
    k32 = np.asarray(k, np.float32).reshape(B * H, S, DK)
    vdt = np.float16 if use_fp16 else ml_dtypes.bfloat16
    # [h, s*128+p, d] -> [h, p, s, d]: contiguous per-partition DMA runs.
    v32 = np.ascontiguousarray(
        np.asarray(v, vdt).reshape(B * H, NSTRIP, P, DK).swapaxes(1, 2)
    )
    if qk_dr:
        qkdt = ml_dtypes.float8_e4m3
    else:
        qkdt = (np.float16 if use_fp16 else ml_dtypes.bfloat16) if qk_bf16 else np.float32
    qT = np.ascontiguousarray(q32.transpose(0, 2, 1)).astype(qkdt)
    kT = np.ascontiguousarray(k32.transpose(0, 2, 1)).astype(qkdt)
    if qk_dr:
        # [h, d, s] -> [h, d//2, 2, s], d = 2p + j
        qT = qT.reshape(B * H, DK // 2, 2, S)
        kT = kT.reshape(B * H, DK // 2, 2, S)
    maskT = np.asarray(mask).reshape(S, S).T            # [k, q]
    mdt = ml_dtypes.float8_e4m3 if mask_fp8 else ml_dtypes.bfloat16
    mTb = np.where(maskT, np.float32(MASK_BIAS), np.float32(0.0)).astype(mdt)
    idr = None
    if mask_dr:
        # [k, q] -> [strip, p, j, q] with k = 128*strip + 2p + j for p < 64;
        # partitions 64-127 zero-padded (HW DoubleRow wants 128 partitions).
        mT_dr = np.zeros((NSTRIP, P, 2, S), mdt)
        mT_dr[:, : P // 2] = mTb.reshape(NSTRIP, P // 2, 2, S)
        mTb = mT_dr
        idr = np.zeros((P, 2, P), mdt)
        for p in range(P // 2):
            for j in range(2):
                idr[p, j, 2 * p + j] = 1.0
    elif mask_quad:
        i64 = np.eye(P // 2).astype(mdt)
        idr = np.concatenate([i64, i64], axis=0)
    in_maps = []
    for c in range(N_CORES):
        sl = slice(c * HPC, (c + 1) * HPC)
        m = {
            "qT": np.ascontiguousarray(qT[sl]),
            "kT": np.ascontiguousarray(kT[sl]),
            "v": v32[sl],
            "mT": mTb,
        }
        if idr is not None:
            m["idr"] = idr
        in_maps.append(m)
    return in_maps


def assemble_out(per_core_outT):
    """8 x [HPC, DK, S] -> [B, H, S, DK]."""
    out = np.concatenate([np.asarray(o) for o in per_core_outT], axis=0)
    return np.ascontiguousarray(
        out.reshape(B, H, DK, S).transpose(0, 1, 3, 2)
    ).astype(np.float32)


def kernel(q, k, v, mask):
    from concourse.bass_utils import run_bass_kernel_spmd

    nc = get_nc()
    in_maps = make_in_maps(q, k, v, mask)
    res = run_bass_kernel_spmd(nc, in_maps, core_ids=list(range(N_CORES)))
    return assemble_out([r["outT"] for r in res.results])

